# revision 12
# baseline (speedup 1.0000x reference)
"""GATv2 QSAR network (3-layer GATv2 + mean-pool + MLP) on 8 Trainium2 cores.

v2.5: bf16 PE datapath; batched one-hot builds on DVE via stride-0 APs;
node-major scatter PSUM (no finalize transpose); next layer's dense fused
into the group finalize; SWDGE gathers merged per super-group (2 dst-groups)
per parity; GPSIMD runs nothing but the gathers.

Sharding: destination-node range per core (6250 nodes + its in-edges).
Per layer:
  dense:  per-core node transforms -> xl tables, parity-split (int16 SWDGE
          indices), bf16, AllGather; xr kept in SBUF.
  edges:  dst-sorted edges in 128-node dst-groups; chunk order interleaves
          super-groups of 2 so same-parity runs merge into one gather call:
          - dma_gather (bf16, 256B rows) per merged run, 4 SWDGE queues
          - batched M2 one-hot [node x edge] via drel_row DMA-broadcast +
            in-place is_equal (1 DMA + 1 DVE op per batch)
          - per chunk: psum m' = M2c.T @ xr_g + I @ slab_c ; ACT Prelu -> L
          - DVE sign-span reduces -> e ; batched ACT Exp -> p
          - batched M1p = (drel==iota)*p (2 DVE ops per batch)
          - per chunk scatter psum_g[128 node, sc] += m1p_c.T @ slab_c
            (table ones-col => softmax denominator)
          - group finalize: divide, bias, relu, transpose, next-layer dense.
|att| is folded into W columns (features sign-sorted); the inverse scaling
and permutation fold into the next layer's weights. Softmax max-subtraction
is skipped (|e| < 7 here; softmax is shift-invariant: a rounding choice).
"""
import os
import numpy as np
import ml_dtypes
from contextlib import ExitStack

from concourse import bacc, mybir as mb, tile, bass
from concourse import library_config
from concourse.bass_utils import run_bass_kernel_spmd

# ---------------- problem constants ----------------
N = 50000
NUM_GRAPHS = 2000
NEG_SLOPE = 0.2
N_CORES = 8
NPC = N // N_CORES                     # 6250 nodes per core
NPAD = 6272                            # 49 * 128
GROUPS = NPAD // 128                   # 49
EV_ROWS = NPAD // 2                    # 3136 rows per core per parity table
D_IN = 27
DIMS = [(D_IN, 100), (100, 60), (60, 30)]
ELEM = 128                             # table row width, bf16 (256B rows)
SC = [128, 64, 64]                     # scatter cols (>= d_out+1)
POOL_ELEM = 32                         # h3 row: 30 feats + count-col + pad
MAXW = 1

f32, bf16, i16 = mb.dt.float32, mb.dt.bfloat16, mb.dt.int16
AF = mb.ActivationFunctionType
OP = mb.AluOpType


def split_excess_waits(nc, maxw=MAXW):
    n_split = 0
    for fn in nc.m.functions:
        for bb in fn.blocks:
            out = []
            for ins in bb.instructions:
                si = ins.sync_info
                waits = list(si.on_wait) if si and si.on_wait else []
                if len(waits) > maxw:
                    extra = waits[maxw:]
                    for ci in range(0, len(extra), maxw):
                        d = mb.InstDrain(name=f"{ins.name}_ws{ci}")
                        d.engine = ins.engine
                        d.sync_info = mb.SyncInfo(on_wait=extra[ci:ci + maxw], on_update=[])
                        out.append(d)
                        n_split += 1
                    ins.sync_info = mb.SyncInfo(
                        on_wait=waits[:maxw],
                        on_update=list(si.on_update) if si.on_update else [])
                out.append(ins)
            bb.instructions = out
    return n_split


# ---------------- host prep ----------------
def fold_layer(Wl, Wr, att):
    a = np.asarray(att, np.float32)
    order = np.argsort(a <= 0, kind="stable")
    s = np.abs(a[order]).astype(np.float32)
    n_pos = int((a > 0).sum())
    Wl_p = (np.asarray(Wl, np.float32)[order] * s[:, None])
    Wr_p = (np.asarray(Wr, np.float32)[order] * s[:, None])
    return Wl_p.astype(np.float32), Wr_p.astype(np.float32), n_pos, order, s


def build_edge_schedule(src, dst):
    """Chunk order: per super-group (2 dst-groups) emit p=0 runs of both
    groups (merged into one gather), then p=1 runs. int16 parity tables."""
    core = dst // NPC
    per_core = {}
    cnt_all = np.zeros((N_CORES, GROUPS, 2), np.int64)
    for k in range(N_CORES):
        m = core == k
        s_k = src[m]
        rel = dst[m] - k * NPC
        grp = rel // 128
        par = (s_k & 1).astype(np.int64)
        o = np.lexsort((par, grp))
        s_k, rel, grp, par = s_k[o], rel[o], grp[o], par[o]
        key = grp * 2 + par
        cnt = np.bincount(key, minlength=GROUPS * 2).reshape(GROUPS, 2)
        cnt_all[k] = cnt
        # start offset of each (g,p) segment in the sorted arrays
        starts = np.zeros(GROUPS * 2 + 1, np.int64)
        np.cumsum(cnt.reshape(-1), out=starts[1:])
        per_core[k] = (s_k, rel, starts)

    nch = (cnt_all.max(axis=0) + 127) // 128          # [GROUPS, 2]

    chunk_grp = []          # group of each chunk
    batches = []            # list of (c0, nB, runs) ; runs = (c_start, n, p)
    run_of = {}             # (g, p) -> chunk start (for slot filling)
    for g0 in range(0, GROUPS, 2):
        pair = [g for g in (g0, g0 + 1) if g < GROUPS]
        b_c0 = len(chunk_grp)
        runs = []
        for p in (0, 1):
            c_start = len(chunk_grp)
            n = 0
            for g in pair:
                run_of[(g, p)] = len(chunk_grp)
                chunk_grp += [g] * int(nch[g, p])
                n += int(nch[g, p])
            if n:
                runs.append((c_start, n, p))
        batches.append((b_c0, len(chunk_grp) - b_c0, runs))
    total_chunks = len(chunk_grp)
    total_slots = total_chunks * 128

    idx_i16 = np.zeros((N_CORES, total_slots), np.int16)
    dstrel = np.full((N_CORES, total_slots), -1.0, np.float32)
    for k in range(N_CORES):
        s_k, rel, starts = per_core[k]
        row16 = ((s_k // NPC) * EV_ROWS + (s_k % NPC) // 2).astype(np.int16)
        for g in range(GROUPS):
            for p in (0, 1):
                c = int(cnt_all[k, g, p])
                if c == 0:
                    continue
                seg = starts[g * 2 + p]
                off = run_of[(g, p)] * 128
                idx_i16[k, off:off + c] = row16[seg:seg + c]
                dstrel[k, off:off + c] = (rel[seg:seg + c] - g * 128).astype(np.float32)
    cap_b = max(nB for (_, nB, _) in batches)
    return dict(nch=nch, total_chunks=total_chunks, total_slots=total_slots,
                chunk_grp=chunk_grp, batches=batches, cap_b=cap_b,
                idx_i16=idx_i16, dstrel=dstrel)


def pack_idx(idx_flat):
    S = idx_flat.shape[0]
    a = idx_flat.reshape(S // 16, 16).T
    return np.tile(a, (8, 1)).astype(np.int16)


def build_pool_schedule(batch):
    rows = N_CORES * NPAD
    g_of_row = np.full(rows, -1, np.int64)
    for k in range(N_CORES):
        g_of_row[k * NPAD: k * NPAD + NPC] = batch[k * NPC:(k + 1) * NPC]
    jobs = []
    for c in range(rows // 128):
        g = g_of_row[c * 128:(c + 1) * 128]
        valid = g >= 0
        if not valid.any():
            continue
        for w in np.unique(g[valid] // 128):
            rel = np.where(valid & (g // 128 == w), g - w * 128, -1).astype(np.float32)
            jobs.append((c, int(w), rel))
    n_win = (NUM_GRAPHS + 127) // 128
    return jobs, n_win


def manual_ap(base_ap, ap_list):
    return bass.AP(base_ap.tensor, base_ap.offset, ap_list)


# ---------------- device program ----------------
def build_program(sched, n_pos_l, n_jobs, n_win):
    total_chunks = sched["total_chunks"]
    total_slots = sched["total_slots"]
    chunk_grp = sched["chunk_grp"]
    batches = sched["batches"]
    cap_b = sched["cap_b"]

    nc = bacc.Bacc("TRN2", target_bir_lowering=False, debug=False,
                   num_devices=N_CORES, num_swdge_queues=4)

    xT = nc.dram_tensor("xT", [D_IN + 1, NPAD], f32, kind="ExternalInput")
    idx_in = nc.dram_tensor("idx", [128, total_slots // 16], i16, kind="ExternalInput")
    drel_col = nc.dram_tensor("drel_col", [128, total_chunks], f32, kind="ExternalInput")
    drel_row = nc.dram_tensor("drel_row", [1, total_slots], bf16, kind="ExternalInput")
    iota_row_in = nc.dram_tensor("iota_row", [128, 128], f32, kind="ExternalInput")
    iota_col_in = nc.dram_tensor("iota_col", [128, 1], f32, kind="ExternalInput")
    Win = {}
    for li in range(3):
        d_in, d_out = DIMS[li]
        Win[f"Wl{li}"] = nc.dram_tensor(f"Wl{li}", [d_in + 1, ELEM], f32, kind="ExternalInput")
        Win[f"Wr{li}"] = nc.dram_tensor(f"Wr{li}", [d_in + 1, d_out], f32, kind="ExternalInput")
        Win[f"b{li}"] = nc.dram_tensor(f"b{li}", [128, d_out], f32, kind="ExternalInput")
    poolrel = nc.dram_tensor("poolrel", [128, max(n_jobs, 1)], f32, kind="ExternalInput")
    Wfc1 = nc.dram_tensor("Wfc1", [POOL_ELEM, 20], f32, kind="ExternalInput")
    bfc1 = nc.dram_tensor("bfc1", [128, 20], f32, kind="ExternalInput")
    Wfc2 = nc.dram_tensor("Wfc2", [32, 1], f32, kind="ExternalInput")
    bfc2 = nc.dram_tensor("bfc2", [128, 1], f32, kind="ExternalInput")
    out_t = nc.dram_tensor("out", [n_win * 128, 1], f32, kind="ExternalOutput")

    first_chunk = {}
    last_chunk = {}
    for c, g in enumerate(chunk_grp):
        if g not in first_chunk:
            first_chunk[g] = c
        last_chunk[g] = c

    with tile.TileContext(nc) as tc, ExitStack() as ctx:
        sbuf = ctx.enter_context(tc.tile_pool(name="sbuf", bufs=3))
        sbslab = ctx.enter_context(tc.tile_pool(name="sbslab", bufs=2))
        sbx = ctx.enter_context(tc.tile_pool(name="sbx", bufs=1))
        psum = ctx.enter_context(tc.tile_pool(name="psum", bufs=2, space="PSUM"))
        psumo = ctx.enter_context(tc.tile_pool(name="psumo", bufs=2, space="PSUM"))
        dram = ctx.enter_context(tc.tile_pool(name="dram", bufs=1, space="DRAM"))
        const = ctx.enter_context(tc.tile_pool(name="const", bufs=1))

        nc.gpsimd.load_library(library_config.mlp)

        iota_row = const.tile([128, 128], f32)
        nc.sync.dma_start(out=iota_row[:], in_=iota_row_in[:])
        iota_col = const.tile([128, 1], f32)
        nc.sync.dma_start(out=iota_col[:], in_=iota_col_in[:])
        ident = const.tile([128, 128], bf16)
        nc.vector.tensor_scalar(out=ident[:], in0=iota_row[:], scalar1=iota_col[:],
                                scalar2=None, op0=OP.is_equal)
        identf = const.tile([128, 128], f32)
        nc.vector.tensor_scalar(out=identf[:], in0=iota_row[:], scalar1=iota_col[:],
                                scalar2=None, op0=OP.is_equal)
        drelc = const.tile([128, total_chunks], f32)
        nc.sync.dma_start(out=drelc[:], in_=drel_col[:])
        idx_t = const.tile([128, total_slots // 16], i16)
        nc.sync.dma_start(out=idx_t[:], in_=idx_in[:])

        W_t = {}
        for li in range(3):
            d_in, d_out = DIMS[li]
            t = const.tile([d_in + 1, ELEM], f32, name=f"wWl{li}", tag=f"wWl{li}")
            nc.sync.dma_start(out=t[:], in_=Win[f"Wl{li}"][:])
            W_t[f"Wl{li}"] = t
            t = const.tile([d_in + 1, d_out], f32, name=f"wWr{li}", tag=f"wWr{li}")
            nc.sync.dma_start(out=t[:], in_=Win[f"Wr{li}"][:])
            W_t[f"Wr{li}"] = t
            t = const.tile([128, d_out], f32, name=f"wb{li}", tag=f"wb{li}")
            nc.sync.dma_start(out=t[:], in_=Win[f"b{li}"][:])
            W_t[f"b{li}"] = t

        # internal DRAM: per-layer parity table shards + gathered tables
        sh_ev, sh_od, tbl_ev, tbl_od = [], [], [], []
        for li in range(3):
            sh_ev.append(dram.tile([EV_ROWS, ELEM], bf16, name=f"shev{li}"))
            sh_od.append(dram.tile([EV_ROWS, ELEM], bf16, name=f"shod{li}"))
            tbl_ev.append(dram.tile([EV_ROWS * N_CORES, ELEM], bf16, name=f"tev{li}",
                                    addr_space="Shared"))
            tbl_od.append(dram.tile([EV_ROWS * N_CORES, ELEM], bf16, name=f"tod{li}",
                                    addr_space="Shared"))
        h3_sh = dram.tile([NPAD, POOL_ELEM], f32)
        h3_full = dram.tile([NPAD * N_CORES, POOL_ELEM], f32, addr_space="Shared")

        # xr tables in SBUF: xr_sb[li][p, g*d_out + j] = xr_j of node g*128+p
        xr_sb = [sbx.tile([128, GROUPS * DIMS[li][1]], bf16, name=f"xr{li}",
                          tag=f"xr{li}") for li in range(3)]

        def allgather(li):
            nc.gpsimd.collective_compute(
                "AllGather", OP.bypass, replica_groups=[list(range(N_CORES))],
                ins=[sh_ev[li][:].opt()], outs=[tbl_ev[li][:].opt()])
            nc.gpsimd.collective_compute(
                "AllGather", OP.bypass, replica_groups=[list(range(N_CORES))],
                ins=[sh_od[li][:].opt()], outs=[tbl_od[li][:].opt()])

        # ---------------- layer-0 dense ----------------
        def dense0():
            d_in, d_out = DIMS[0]
            for g in range(GROUPS):
                lhs = sbuf.tile([d_in + 1, 128], f32, tag="dlhs")
                nc.sync.dma_start(out=lhs[:], in_=xT[:, g * 128:(g + 1) * 128])
                pm = psum.tile([128, ELEM], f32, tag="dns", space="PSUM")
                nc.tensor.matmul(out=pm[:], lhsT=lhs[:], rhs=W_t["Wl0"][:],
                                 start=True, stop=True)
                esc = sbuf.tile([128, ELEM], bf16, tag="desc")
                if g % 2 == 0:
                    nc.vector.tensor_copy(out=esc[:], in_=pm[:])
                else:
                    nc.scalar.copy(out=esc[:], in_=pm[:])
                r0 = g * 64
                nc.sync.dma_start(out=sh_ev[0][r0:r0 + 64, :], in_=esc[0:128:2, :])
                nc.sync.dma_start(out=sh_od[0][r0:r0 + 64, :], in_=esc[1:128:2, :])
                pm2 = psum.tile([128, d_out], f32, tag="dns", space="PSUM")
                nc.tensor.matmul(out=pm2[:], lhsT=lhs[:], rhs=W_t["Wr0"][:],
                                 start=True, stop=True)
                if g % 2 == 0:
                    nc.scalar.copy(out=xr_sb[0][:, g * d_out:(g + 1) * d_out], in_=pm2[:])
                else:
                    nc.vector.tensor_copy(out=xr_sb[0][:, g * d_out:(g + 1) * d_out], in_=pm2[:])
            allgather(0)

        # ---------------- edge phase for layer li ----------------
        def edge_phase(li):
            d_in, d_out = DIMS[li]
            sc = SC[li]
            n_pos = n_pos_l[li]
            po_of_group = {}
            qn = [0]

            def finalize_group(g):
                po2 = po_of_group.pop(g)
                dcl = sbuf.tile([128, 1], f32, tag="dcl")
                nc.vector.tensor_scalar(out=dcl[:], in0=po2[:, d_out:d_out + 1],
                                        scalar1=1e-30, scalar2=None, op0=OP.max)
                rec = sbuf.tile([128, 1], f32, tag="rec")
                nc.vector.reciprocal(out=rec[:], in_=dcl[:])
                hg = sbuf.tile([128, POOL_ELEM if li == 2 else d_out], f32, tag="hg")
                nc.vector.tensor_scalar(out=hg[:, :d_out], in0=po2[:, :d_out],
                                        scalar1=rec[:], scalar2=None, op0=OP.mult)
                nc.vector.tensor_tensor(out=hg[:, :d_out], in0=hg[:, :d_out],
                                        in1=W_t[f"b{li}"][:], op=OP.add)
                if li < 2:
                    # relu, transpose, fused dense for layer li+1
                    d_next = DIMS[li + 1][1]
                    h2 = sbuf.tile([128, d_out], f32, tag="hrelu")
                    nc.scalar.activation(out=h2[:], in_=hg[:, :d_out], func=AF.Relu)
                    ptb = psum.tile([d_out, 128], f32, tag="aux", space="PSUM")
                    nc.tensor.transpose(out=ptb[:], in_=h2[:], identity=identf[:])
                    hTe = sbuf.tile([d_out + 1, 128], f32, tag="hTe")
                    nc.vector.memset(hTe[:], 1.0)
                    nc.vector.tensor_copy(out=hTe[:d_out, :], in_=ptb[:])
                    pm = psum.tile([128, ELEM], f32, tag="dns", space="PSUM")
                    nc.tensor.matmul(out=pm[:], lhsT=hTe[:], rhs=W_t[f"Wl{li + 1}"][:],
                                     start=True, stop=True)
                    esc = sbuf.tile([128, ELEM], bf16, tag="desc")
                    if g % 2 == 0:
                        nc.vector.tensor_copy(out=esc[:], in_=pm[:])
                    else:
                        nc.scalar.copy(out=esc[:], in_=pm[:])
                    r0 = g * 64
                    nc.sync.dma_start(out=sh_ev[li + 1][r0:r0 + 64, :], in_=esc[0:128:2, :])
                    nc.sync.dma_start(out=sh_od[li + 1][r0:r0 + 64, :], in_=esc[1:128:2, :])
                    pm2 = psum.tile([128, d_next], f32, tag="dns", space="PSUM")
                    nc.tensor.matmul(out=pm2[:], lhsT=hTe[:], rhs=W_t[f"Wr{li + 1}"][:],
                                     start=True, stop=True)
                    if g % 2 == 0:
                        nc.scalar.copy(out=xr_sb[li + 1][:, g * d_next:(g + 1) * d_next],
                                       in_=pm2[:])
                    else:
                        nc.vector.tensor_copy(out=xr_sb[li + 1][:, g * d_next:(g + 1) * d_next],
                                              in_=pm2[:])
                else:
                    nc.vector.memset(hg[:, d_out:d_out + 1], 1.0)
                    nc.vector.memset(hg[:, d_out + 1:], 0.0)
                    nc.sync.dma_start(out=h3_sh[g * 128:(g + 1) * 128, :], in_=hg[:])

            for (c0, nB, runs) in batches:
                # 1) gather: one SWDGE call per merged same-parity run
                slab = sbslab.tile([128, cap_b * ELEM], bf16, tag="slab")
                for (cs, n, p) in runs:
                    boff = cs - c0
                    tbl = tbl_ev[li] if p == 0 else tbl_od[li]
                    nc.gpsimd.dma_gather(
                        out_ap=slab[:, boff * ELEM:(boff + n) * ELEM]
                        .rearrange("q (c e) -> q c e", e=ELEM),
                        in_ap=tbl[:], idxs_ap=idx_t[:, cs * 8:(cs + n) * 8],
                        num_idxs=n * 128, num_idxs_reg=n * 128,
                        elem_size=ELEM, single_packet=False, queue_num=qn[0] % 4)
                    qn[0] += 1
                # 2) batched M2 one-hot [node, edge]: DMA partition-broadcast
                #    of drel_row, then in-place is_equal against iota_col
                m2 = sbslab.tile([128, cap_b * 128], bf16, tag="m2")
                nc.sync.dma_start(
                    out=m2[:, :nB * 128],
                    in_=drel_row[0:1, c0 * 128:(c0 + nB) * 128]
                    .to_broadcast([128, nB * 128]))
                nc.vector.tensor_scalar(
                    out=m2[:, :nB * 128], in0=m2[:, :nB * 128],
                    scalar1=iota_col[:], scalar2=None, op0=OP.is_equal)
                # 3) m' + Prelu + reduces per 4-chunk sub-slab
                eep = sbuf.tile([128, cap_b], f32, tag="eep")
                een = sbuf.tile([128, cap_b], f32, tag="een")
                for s0 in range(0, nB, 4):
                    cn = min(4, nB - s0)
                    pm = psum.tile([128, 4 * d_out], f32, tag="pm", space="PSUM")
                    for j in range(cn):
                        c = c0 + s0 + j
                        g = chunk_grp[c]
                        nc.tensor.matmul(
                            out=pm[:, j * d_out:(j + 1) * d_out],
                            lhsT=m2[:, (s0 + j) * 128:(s0 + j + 1) * 128],
                            rhs=xr_sb[li][:, g * d_out:(g + 1) * d_out],
                            start=True, stop=False)
                        nc.tensor.matmul(
                            out=pm[:, j * d_out:(j + 1) * d_out],
                            lhsT=ident[:],
                            rhs=slab[:, (s0 + j) * ELEM:(s0 + j) * ELEM + d_out],
                            start=False, stop=True)
                    Ls = sbuf.tile([128, 4 * d_out], bf16, tag="lslab")
                    nc.scalar.activation(
                        out=Ls[:, :cn * d_out],
                        in_=pm[:, :cn * d_out], func=AF.Prelu, alpha=NEG_SLOPE)
                    Lv = Ls[:, :cn * d_out].rearrange("q (c d) -> q c d", d=d_out)
                    if n_pos > 0:
                        nc.vector.tensor_reduce(out=eep[:, s0:s0 + cn], in_=Lv[:, :, :n_pos],
                                                axis=mb.AxisListType.X, op=OP.add)
                    else:
                        nc.vector.memset(eep[:, s0:s0 + cn], 0.0)
                    if n_pos < d_out:
                        nc.vector.tensor_reduce(out=een[:, s0:s0 + cn], in_=Lv[:, :, n_pos:],
                                                axis=mb.AxisListType.X, op=OP.add)
                    else:
                        nc.vector.memset(een[:, s0:s0 + cn], 0.0)
                ee = sbuf.tile([128, cap_b], f32, tag="ee")
                nc.vector.tensor_tensor(out=ee[:, :nB], in0=eep[:, :nB],
                                        in1=een[:, :nB], op=OP.subtract)
                pe = sbuf.tile([128, cap_b], f32, tag="pe")
                nc.scalar.activation(out=pe[:, :nB], in_=ee[:, :nB], func=AF.Exp)
                # 4) batched M1p = (iota == drel) * p   [edge, node]
                m1p = sbslab.tile([128, cap_b * 128], bf16, tag="m1p")
                iota_b = manual_ap(iota_row[:], [list(iota_row[:].ap[0]), [0, nB],
                                                 list(iota_row[:].ap[1])])
                nc.vector.tensor_tensor(
                    out=m1p[:, :nB * 128].rearrange("q (c n) -> q c n", n=128),
                    in0=drelc[:, c0:c0 + nB].to_broadcast([128, nB, 128]),
                    in1=iota_b, op=OP.is_equal)
                nc.vector.tensor_tensor(
                    out=m1p[:, :nB * 128].rearrange("q (c n) -> q c n", n=128),
                    in0=m1p[:, :nB * 128].rearrange("q (c n) -> q c n", n=128),
                    in1=pe[:, :nB].to_broadcast([128, nB, 128]), op=OP.mult)
                # 5) scatter + finalize
                for b in range(nB):
                    c = c0 + b
                    g = chunk_grp[c]
                    if c == first_chunk[g]:
                        po2 = psumo.tile([128, sc], f32, tag="pout",
                                         space="PSUM", name=f"po_{li}_{g}")
                        po_of_group[g] = po2
                    nc.tensor.matmul(
                        out=po_of_group[g][:],
                        lhsT=m1p[:, b * 128:(b + 1) * 128],
                        rhs=slab[:, b * ELEM:b * ELEM + sc],
                        start=(c == first_chunk[g]), stop=(c == last_chunk[g]))
                    if c == last_chunk[g]:
                        finalize_group(g)

            if li < 2:
                allgather(li + 1)
            else:
                nc.gpsimd.collective_compute(
                    "AllGather", OP.bypass, replica_groups=[list(range(N_CORES))],
                    ins=[h3_sh[:].opt()], outs=[h3_full[:].opt()])

        # ---------------- emit ----------------
        dense0()
        for li in range(3):
            edge_phase(li)

        # ---------------- pooling + MLP (replicated on every core) ----------------
        poolrel_t = const.tile([128, max(n_jobs, 1)], f32)
        nc.sync.dma_start(out=poolrel_t[:], in_=poolrel[:])
        wfc1_t = const.tile([POOL_ELEM, 20], f32)
        nc.sync.dma_start(out=wfc1_t[:], in_=Wfc1[:])
        bfc1_t = const.tile([128, 20], f32)
        nc.sync.dma_start(out=bfc1_t[:], in_=bfc1[:])
        wfc2_t = const.tile([32, 1], f32)
        nc.sync.dma_start(out=wfc2_t[:], in_=Wfc2[:])
        bfc2_t = const.tile([128, 1], f32)
        nc.sync.dma_start(out=bfc2_t[:], in_=bfc2[:])

        from collections import defaultdict
        by_win = defaultdict(list)
        for j, (chunk, w) in enumerate(_POOL_JOBS_META):
            by_win[w].append((j, chunk))
        for w in range(n_win):
            jobs_w = by_win.get(w, [])
            pw = psumo.tile([POOL_ELEM, 128], f32, tag="pout", space="PSUM")
            if not jobs_w:
                o1 = sbuf.tile([128, 1], f32, tag="ow")
                nc.vector.memset(o1[:], 0.0)
                nc.sync.dma_start(out=out_t[w * 128:(w + 1) * 128, :], in_=o1[:])
                continue
            for jj, (j, chunk) in enumerate(jobs_w):
                hch = sbuf.tile([128, POOL_ELEM], f32, tag="hch")
                nc.sync.dma_start(out=hch[:], in_=h3_full[chunk * 128:(chunk + 1) * 128, :])
                oh = sbuf.tile([128, 128], f32, tag="poh")
                nc.vector.tensor_scalar(out=oh[:], in0=iota_row[:],
                                        scalar1=poolrel_t[:, j:j + 1], scalar2=None,
                                        op0=OP.is_equal)
                nc.tensor.matmul(out=pw[:], lhsT=hch[:], rhs=oh[:],
                                 start=(jj == 0), stop=(jj == len(jobs_w) - 1))
            pesc = sbuf.tile([POOL_ELEM, 128], f32, tag="pesc")
            nc.scalar.copy(out=pesc[:], in_=pw[:])
            ptw = psum.tile([128, POOL_ELEM], f32, tag="aux", space="PSUM")
            nc.tensor.transpose(out=ptw[:], in_=pesc[:], identity=identf[:POOL_ELEM, :POOL_ELEM])
            cnt_r = sbuf.tile([128, 1], f32, tag="cntr")
            nc.vector.tensor_scalar(out=cnt_r[:], in0=ptw[:, 30:31], scalar1=1.0,
                                    scalar2=None, op0=OP.max)
            rec = sbuf.tile([128, 1], f32, tag="prec")
            nc.vector.reciprocal(out=rec[:], in_=cnt_r[:])
            gt = sbuf.tile([128, POOL_ELEM], f32, tag="gt")
            nc.vector.tensor_scalar(out=gt[:], in0=ptw[:], scalar1=rec[:],
                                    scalar2=None, op0=OP.mult)
            nc.vector.memset(gt[:, 30:], 0.0)
            pgt = psum.tile([POOL_ELEM, 128], f32, tag="aux", space="PSUM")
            nc.tensor.transpose(out=pgt[:], in_=gt[:], identity=identf[:])
            gT = sbuf.tile([POOL_ELEM, 128], f32, tag="gTt")
            nc.vector.tensor_copy(out=gT[:], in_=pgt[:])
            pf1 = psum.tile([128, 20], f32, tag="aux", space="PSUM")
            nc.tensor.matmul(out=pf1[:], lhsT=gT[:], rhs=wfc1_t[:], start=True, stop=True)
            g1 = sbuf.tile([128, 32], f32, tag="g1")
            nc.vector.tensor_tensor(out=g1[:, :20], in0=pf1[:], in1=bfc1_t[:], op=OP.add)
            g1r = sbuf.tile([128, 32], f32, tag="g1r")
            nc.scalar.activation(out=g1r[:, :20], in_=g1[:, :20], func=AF.Relu)
            nc.vector.memset(g1r[:, 20:], 0.0)
            pg1 = psum.tile([32, 128], f32, tag="aux", space="PSUM")
            nc.tensor.transpose(out=pg1[:], in_=g1r[:], identity=identf[:])
            g1T = sbuf.tile([32, 128], f32, tag="g1T")
            nc.vector.tensor_copy(out=g1T[:], in_=pg1[:])
            pf2 = psum.tile([128, 1], f32, tag="aux", space="PSUM")
            nc.tensor.matmul(out=pf2[:], lhsT=g1T[:], rhs=wfc2_t[:], start=True, stop=True)
            ow = sbuf.tile([128, 1], f32, tag="ow")
            nc.vector.tensor_tensor(out=ow[:], in0=pf2[:], in1=bfc2_t[:], op=OP.add)
            nc.sync.dma_start(out=out_t[w * 128:(w + 1) * 128, :], in_=ow[:])

    return nc


_POOL_JOBS_META = []


# ---------------- top-level kernel ----------------
_CACHE = {}


def _install_ntff_hook():
    """Make trace=True work under axon when antenv.axon_hooks is missing."""
    import sys, types
    try:
        from antenv.axon_hooks import get_axon_ntff_profile_hook  # noqa
        return
    except ImportError:
        pass
    try:
        mod = types.ModuleType("antenv.axon_hooks")
        mod._hook = None
        mod.set_axon_ntff_profile_hook = lambda h: setattr(mod, "_hook", h)
        mod.get_axon_ntff_profile_hook = lambda: mod._hook
        try:
            import antenv
            antenv.axon_hooks = mod
        except ImportError:
            pkg = types.ModuleType("antenv")
            pkg.axon_hooks = mod
            sys.modules["antenv"] = pkg
        sys.modules["antenv.axon_hooks"] = mod
        from trn_agent_boot.trn_boot import _ntff_profile_via_ctypes
        mod.set_axon_ntff_profile_hook(_ntff_profile_via_ctypes('/opt/axon/libaxon_pjrt.so'))
        import concourse.bass_utils as bu
        bu.upload_artifacts = lambda d: str(d)
    except Exception as e:
        print("ntff hook install failed:", e)


def kernel(**inputs):
    global _POOL_JOBS_META
    x = np.asarray(inputs["x"], np.float32)
    ei = np.asarray(inputs["edge_index"], np.int64)
    batch = np.asarray(inputs["batch"], np.int64)

    loops = np.arange(N, dtype=np.int64)
    src = np.concatenate([ei[0], loops])
    dst = np.concatenate([ei[1], loops])

    sched = build_edge_schedule(src, dst)
    pool_jobs, n_win = build_pool_schedule(batch)
    pool_jobs.sort(key=lambda t: (t[1], t[0]))
    _POOL_JOBS_META = [(c, w) for (c, w, _) in pool_jobs]

    # ---- fold weights ----
    n_pos_l = []
    Wmats = {}
    prev_order, prev_s = None, None
    for li in range(3):
        d_in, d_out = DIMS[li]
        Wl, Wr, n_pos, order, s = fold_layer(inputs[f"Wl{li + 1}"], inputs[f"Wr{li + 1}"],
                                             inputs[f"att{li + 1}"])
        if prev_order is not None:
            Wl = (Wl[:, prev_order] / prev_s[None, :]).astype(np.float32)
            Wr = (Wr[:, prev_order] / prev_s[None, :]).astype(np.float32)
        n_pos_l.append(n_pos)
        b_t = (s * np.asarray(inputs[f"b{li + 1}"], np.float32)[order]).astype(np.float32)
        # augmented: [d_in+1, ELEM] ; last input row = ones channel -> table ones col
        Wa = np.zeros((d_in + 1, ELEM), np.float32)
        Wa[:d_in, :d_out] = Wl.T
        Wa[d_in, d_out] = 1.0            # ones column for denominators
        Wra = np.zeros((d_in + 1, d_out), np.float32)
        Wra[:d_in, :] = Wr.T
        Wmats[f"Wl{li}"] = Wa
        Wmats[f"Wr{li}"] = Wra
        Wmats[f"b{li}"] = np.tile(b_t[None, :], (128, 1)).astype(np.float32)
        prev_order, prev_s = order, s

    # FC weights; fold layer-3 unscale/perm into W_fc1
    Wfc1 = np.asarray(inputs["W_fc1"], np.float32)          # [20, 30]
    Wfc1_f = (Wfc1[:, prev_order] / prev_s[None, :]).astype(np.float32)
    Wfc1_a = np.zeros((POOL_ELEM, 20), np.float32)
    Wfc1_a[:30, :] = Wfc1_f.T
    bfc1 = np.tile(np.asarray(inputs["b_fc1"], np.float32)[None, :], (128, 1))
    Wfc2_a = np.zeros((32, 1), np.float32)
    Wfc2_a[:20, 0] = np.asarray(inputs["W_fc2"], np.float32)[0]
    bfc2 = np.full((128, 1), float(np.asarray(inputs["b_fc2"], np.float32)[0]), np.float32)

    # ---- per-core inputs ----
    iota_row = np.broadcast_to(np.arange(128, dtype=np.float32), (128, 128)).copy()
    iota_col = np.arange(128, dtype=np.float32)[:, None].copy()
    poolrel = np.zeros((128, max(len(pool_jobs), 1)), np.float32)
    for j, (_, _, rel) in enumerate(pool_jobs):
        poolrel[:, j] = rel

    TC = sched["total_chunks"]
    in_maps = []
    for k in range(N_CORES):
        xTl = np.zeros((D_IN + 1, NPAD), np.float32)
        xTl[:D_IN, :NPC] = x[k * NPC:(k + 1) * NPC].T
        xTl[D_IN, :NPC] = 1.0
        drel_k = sched["dstrel"][k]
        in_maps.append({
            "xT": xTl,
            "idx": pack_idx(sched["idx_i16"][k]),
            "drel_col": drel_k.reshape(TC, 128).T.copy(),
            "drel_row": drel_k[None, :].astype(ml_dtypes.bfloat16),
            "iota_row": iota_row, "iota_col": iota_col,
            "poolrel": poolrel,
            "Wfc1": Wfc1_a, "bfc1": bfc1, "Wfc2": Wfc2_a, "bfc2": bfc2,
            **{k2: v for k2, v in Wmats.items()},
        })

    key = "prog"
    if key not in _CACHE:
        nc = build_program(sched, n_pos_l, len(pool_jobs), n_win)
        nc.compile()
        split_excess_waits(nc)
        _CACHE[key] = nc
    nc = _CACHE[key]

    if os.environ.get("GAT_BUILD_ONLY", "0") == "1":
        return np.zeros((NUM_GRAPHS, 1), np.float32)
    trace = os.environ.get("GAT_TRACE", "0") == "1"
    if trace:
        _install_ntff_hook()
    r = run_bass_kernel_spmd(nc, in_maps, core_ids=list(range(N_CORES)), trace=trace)
    if trace and r.exec_time_ns is not None:
        print(f"HW exec time: {r.exec_time_ns} ns")
    out = r.results[0]["out"][:NUM_GRAPHS, :].astype(np.float32)
    return out


# revision 14
# speedup vs baseline: 1.2436x; 1.2436x over previous
"""GATv2 QSAR network (3-layer GATv2 + mean-pool + MLP) on 8 Trainium2 cores.

v2.5: bf16 PE datapath; batched one-hot builds on DVE via stride-0 APs;
node-major scatter PSUM (no finalize transpose); next layer's dense fused
into the group finalize; SWDGE gathers merged per super-group (2 dst-groups)
per parity; GPSIMD runs nothing but the gathers.

Sharding: destination-node range per core (6250 nodes + its in-edges).
Per layer:
  dense:  per-core node transforms -> xl tables, parity-split (int16 SWDGE
          indices), bf16, AllGather; xr kept in SBUF.
  edges:  dst-sorted edges in 128-node dst-groups; chunk order interleaves
          super-groups of 2 so same-parity runs merge into one gather call:
          - dma_gather (bf16, 256B rows) per merged run, 4 SWDGE queues
          - batched M2 one-hot [node x edge] via drel_row DMA-broadcast +
            in-place is_equal (1 DMA + 1 DVE op per batch)
          - per chunk: psum m' = M2c.T @ xr_g + I @ slab_c ; ACT Prelu -> L
          - DVE sign-span reduces -> e ; batched ACT Exp -> p
          - batched M1p = (drel==iota)*p (2 DVE ops per batch)
          - per chunk scatter psum_g[128 node, sc] += m1p_c.T @ slab_c
            (table ones-col => softmax denominator)
          - group finalize: divide, bias, relu, transpose, next-layer dense.
|att| is folded into W columns (features sign-sorted); the inverse scaling
and permutation fold into the next layer's weights. Softmax max-subtraction
is skipped (|e| < 7 here; softmax is shift-invariant: a rounding choice).
"""
import os
import numpy as np
import ml_dtypes
from contextlib import ExitStack

from concourse import bacc, mybir as mb, tile, bass
from concourse import library_config
from concourse.bass_utils import run_bass_kernel_spmd

# ---------------- problem constants ----------------
N = 50000
NUM_GRAPHS = 2000
NEG_SLOPE = 0.2
N_CORES = 8
NPC = N // N_CORES                     # 6250 nodes per core
NPAD = 6272                            # 49 * 128
GROUPS = NPAD // 128                   # 49
EV_ROWS = NPAD // 2                    # 3136 rows per core per parity table
D_IN = 27
DIMS = [(D_IN, 100), (100, 60), (60, 30)]
ELEM = 128                             # table row width, bf16 (256B rows)
SC = [128, 64, 64]                     # scatter cols (>= d_out+1)
POOL_ELEM = 32                         # h3 row: 30 feats + count-col + pad
MAXW = 1

f32, bf16, i16 = mb.dt.float32, mb.dt.bfloat16, mb.dt.int16
AF = mb.ActivationFunctionType
OP = mb.AluOpType


def split_excess_waits(nc, maxw=MAXW):
    n_split = 0
    for fn in nc.m.functions:
        for bb in fn.blocks:
            out = []
            for ins in bb.instructions:
                si = ins.sync_info
                waits = list(si.on_wait) if si and si.on_wait else []
                if len(waits) > maxw:
                    extra = waits[maxw:]
                    for ci in range(0, len(extra), maxw):
                        d = mb.InstDrain(name=f"{ins.name}_ws{ci}")
                        d.engine = ins.engine
                        d.sync_info = mb.SyncInfo(on_wait=extra[ci:ci + maxw], on_update=[])
                        out.append(d)
                        n_split += 1
                    ins.sync_info = mb.SyncInfo(
                        on_wait=waits[:maxw],
                        on_update=list(si.on_update) if si.on_update else [])
                out.append(ins)
            bb.instructions = out
    return n_split


# ---------------- host prep ----------------
def fold_layer(Wl, Wr, att):
    """tau-fold: pos cols scaled s; neg cols scaled -NEG_SLOPE*s so a single
    add-reduce over Prelu(0.2)|Prelu(5) blocks gives e. invs undoes the
    column scale at finalize (before relu)."""
    a = np.asarray(att, np.float32)
    order = np.argsort(a <= 0, kind="stable")
    s = np.abs(a[order]).astype(np.float32)
    n_pos = int((a > 0).sum())
    cvec = s.copy()
    cvec[n_pos:] *= -NEG_SLOPE
    Wl_p = (np.asarray(Wl, np.float32)[order] * cvec[:, None])
    Wr_p = (np.asarray(Wr, np.float32)[order] * cvec[:, None])
    return Wl_p.astype(np.float32), Wr_p.astype(np.float32), n_pos, order, cvec


def build_edge_schedule(src, dst):
    """Chunk order: per super-group (2 dst-groups) emit p=0 runs of both
    groups (merged into one gather), then p=1 runs. int16 parity tables."""
    core = dst // NPC
    per_core = {}
    cnt_all = np.zeros((N_CORES, GROUPS, 2), np.int64)
    for k in range(N_CORES):
        m = core == k
        s_k = src[m]
        rel = dst[m] - k * NPC
        grp = rel // 128
        par = (s_k & 1).astype(np.int64)
        o = np.lexsort((par, grp))
        s_k, rel, grp, par = s_k[o], rel[o], grp[o], par[o]
        key = grp * 2 + par
        cnt = np.bincount(key, minlength=GROUPS * 2).reshape(GROUPS, 2)
        cnt_all[k] = cnt
        # start offset of each (g,p) segment in the sorted arrays
        starts = np.zeros(GROUPS * 2 + 1, np.int64)
        np.cumsum(cnt.reshape(-1), out=starts[1:])
        per_core[k] = (s_k, rel, starts)

    nch = (cnt_all.max(axis=0) + 127) // 128          # [GROUPS, 2]

    chunk_grp = []          # group of each chunk
    batches = []            # list of (c0, nB, runs) ; runs = (c_start, n, p)
    run_of = {}             # (g, p) -> chunk start (for slot filling)
    for g0 in range(0, GROUPS, 2):
        pair = [g for g in (g0, g0 + 1) if g < GROUPS]
        b_c0 = len(chunk_grp)
        runs = []
        for p in (0, 1):
            c_start = len(chunk_grp)
            n = 0
            for g in pair:
                run_of[(g, p)] = len(chunk_grp)
                chunk_grp += [g] * int(nch[g, p])
                n += int(nch[g, p])
            if n:
                runs.append((c_start, n, p))
        batches.append((b_c0, len(chunk_grp) - b_c0, runs))
    total_chunks = len(chunk_grp)
    total_slots = total_chunks * 128

    idx_i16 = np.zeros((N_CORES, total_slots), np.int16)
    dstrel = np.full((N_CORES, total_slots), -1.0, np.float32)
    for k in range(N_CORES):
        s_k, rel, starts = per_core[k]
        row16 = ((s_k // NPC) * EV_ROWS + (s_k % NPC) // 2).astype(np.int16)
        for g in range(GROUPS):
            for p in (0, 1):
                c = int(cnt_all[k, g, p])
                if c == 0:
                    continue
                seg = starts[g * 2 + p]
                off = run_of[(g, p)] * 128
                idx_i16[k, off:off + c] = row16[seg:seg + c]
                dstrel[k, off:off + c] = (rel[seg:seg + c] - g * 128).astype(np.float32)
    cap_b = max(nB for (_, nB, _) in batches)
    return dict(nch=nch, total_chunks=total_chunks, total_slots=total_slots,
                chunk_grp=chunk_grp, batches=batches, cap_b=cap_b,
                idx_i16=idx_i16, dstrel=dstrel)


def pack_idx(idx_flat):
    S = idx_flat.shape[0]
    a = idx_flat.reshape(S // 16, 16).T
    return np.tile(a, (8, 1)).astype(np.int16)


def build_pool_schedule(batch):
    rows = N_CORES * NPAD
    g_of_row = np.full(rows, -1, np.int64)
    for k in range(N_CORES):
        g_of_row[k * NPAD: k * NPAD + NPC] = batch[k * NPC:(k + 1) * NPC]
    jobs = []
    for c in range(rows // 128):
        g = g_of_row[c * 128:(c + 1) * 128]
        valid = g >= 0
        if not valid.any():
            continue
        for w in np.unique(g[valid] // 128):
            rel = np.where(valid & (g // 128 == w), g - w * 128, -1).astype(np.float32)
            jobs.append((c, int(w), rel))
    n_win = (NUM_GRAPHS + 127) // 128
    return jobs, n_win


def manual_ap(base_ap, ap_list):
    return bass.AP(base_ap.tensor, base_ap.offset, ap_list)


# ---------------- device program ----------------
def build_program(sched, n_pos_l, n_jobs, n_win):
    total_chunks = sched["total_chunks"]
    total_slots = sched["total_slots"]
    chunk_grp = sched["chunk_grp"]
    batches = sched["batches"]
    cap_b = sched["cap_b"]

    nc = bacc.Bacc("TRN2", target_bir_lowering=False, debug=False,
                   num_devices=N_CORES, num_swdge_queues=4)

    xT = nc.dram_tensor("xT", [D_IN + 1, NPAD], f32, kind="ExternalInput")
    idx_in = nc.dram_tensor("idx", [128, total_slots // 16], i16, kind="ExternalInput")
    drel_col = nc.dram_tensor("drel_col", [128, total_chunks], bf16, kind="ExternalInput")
    drel_row = nc.dram_tensor("drel_row", [1, total_slots], bf16, kind="ExternalInput")
    iota_row_in = nc.dram_tensor("iota_row", [128, 128], f32, kind="ExternalInput")
    iota_col_in = nc.dram_tensor("iota_col", [128, 1], f32, kind="ExternalInput")
    Win = {}
    for li in range(3):
        d_in, d_out = DIMS[li]
        Win[f"Wl{li}"] = nc.dram_tensor(f"Wl{li}", [d_in + 1, ELEM], f32, kind="ExternalInput")
        Win[f"Wr{li}"] = nc.dram_tensor(f"Wr{li}", [d_in + 1, d_out], f32, kind="ExternalInput")
        Win[f"b{li}"] = nc.dram_tensor(f"b{li}", [128, d_out], f32, kind="ExternalInput")
        Win[f"invs{li}"] = nc.dram_tensor(f"invs{li}", [128, d_out], f32, kind="ExternalInput")
    poolrel = nc.dram_tensor("poolrel", [128, max(n_jobs, 1)], f32, kind="ExternalInput")
    Wfc1 = nc.dram_tensor("Wfc1", [POOL_ELEM, 20], f32, kind="ExternalInput")
    bfc1 = nc.dram_tensor("bfc1", [128, 20], f32, kind="ExternalInput")
    Wfc2 = nc.dram_tensor("Wfc2", [32, 1], f32, kind="ExternalInput")
    bfc2 = nc.dram_tensor("bfc2", [128, 1], f32, kind="ExternalInput")
    out_t = nc.dram_tensor("out", [n_win * 128, 1], f32, kind="ExternalOutput")

    first_chunk = {}
    last_chunk = {}
    for c, g in enumerate(chunk_grp):
        if g not in first_chunk:
            first_chunk[g] = c
        last_chunk[g] = c

    with tile.TileContext(nc) as tc, ExitStack() as ctx:
        sbuf = ctx.enter_context(tc.tile_pool(name="sbuf", bufs=3))
        sbslab = ctx.enter_context(tc.tile_pool(name="sbslab", bufs=2))
        sbgat = ctx.enter_context(tc.tile_pool(name="sbgat", bufs=3))
        sbx = ctx.enter_context(tc.tile_pool(name="sbx", bufs=1))
        psum = ctx.enter_context(tc.tile_pool(name="psum", bufs=2, space="PSUM"))
        psumo = ctx.enter_context(tc.tile_pool(name="psumo", bufs=2, space="PSUM"))
        dram = ctx.enter_context(tc.tile_pool(name="dram", bufs=1, space="DRAM"))
        const = ctx.enter_context(tc.tile_pool(name="const", bufs=1))

        nc.gpsimd.load_library(library_config.mlp)

        iota_row = const.tile([128, 128], f32)
        nc.sync.dma_start(out=iota_row[:], in_=iota_row_in[:])
        iota_col = const.tile([128, 1], f32)
        nc.sync.dma_start(out=iota_col[:], in_=iota_col_in[:])
        ident = const.tile([128, 128], bf16)
        nc.vector.tensor_scalar(out=ident[:], in0=iota_row[:], scalar1=iota_col[:],
                                scalar2=None, op0=OP.is_equal)
        identf = const.tile([128, 128], f32)
        nc.vector.tensor_scalar(out=identf[:], in0=iota_row[:], scalar1=iota_col[:],
                                scalar2=None, op0=OP.is_equal)
        drelc = const.tile([128, total_chunks], bf16)
        nc.sync.dma_start(out=drelc[:], in_=drel_col[:])
        iota16 = const.tile([128, 128], bf16)
        nc.vector.tensor_copy(out=iota16[:], in_=iota_row[:])
        idx_t = const.tile([128, total_slots // 16], i16)
        nc.sync.dma_start(out=idx_t[:], in_=idx_in[:])

        W_t = {}
        for li in range(3):
            d_in, d_out = DIMS[li]
            t = const.tile([d_in + 1, ELEM], f32, name=f"wWl{li}", tag=f"wWl{li}")
            nc.sync.dma_start(out=t[:], in_=Win[f"Wl{li}"][:])
            W_t[f"Wl{li}"] = t
            t = const.tile([d_in + 1, d_out], f32, name=f"wWr{li}", tag=f"wWr{li}")
            nc.sync.dma_start(out=t[:], in_=Win[f"Wr{li}"][:])
            W_t[f"Wr{li}"] = t
            t = const.tile([128, d_out], f32, name=f"wb{li}", tag=f"wb{li}")
            nc.sync.dma_start(out=t[:], in_=Win[f"b{li}"][:])
            W_t[f"b{li}"] = t
            t = const.tile([128, d_out], f32, name=f"winvs{li}", tag=f"winvs{li}")
            nc.sync.dma_start(out=t[:], in_=Win[f"invs{li}"][:])
            W_t[f"invs{li}"] = t

        # internal DRAM: per-layer parity table shards + gathered tables
        sh_ev, sh_od, tbl_ev, tbl_od = [], [], [], []
        for li in range(3):
            sh_ev.append(dram.tile([EV_ROWS, ELEM], bf16, name=f"shev{li}"))
            sh_od.append(dram.tile([EV_ROWS, ELEM], bf16, name=f"shod{li}"))
            tbl_ev.append(dram.tile([EV_ROWS * N_CORES, ELEM], bf16, name=f"tev{li}",
                                    addr_space="Shared"))
            tbl_od.append(dram.tile([EV_ROWS * N_CORES, ELEM], bf16, name=f"tod{li}",
                                    addr_space="Shared"))
        h3_sh = dram.tile([NPAD, POOL_ELEM], f32)
        h3_full = dram.tile([NPAD * N_CORES, POOL_ELEM], f32, addr_space="Shared")

        # xr tables in SBUF: xr_sb[li][p, g*d_out + j] = xr_j of node g*128+p
        xr_sb = [sbx.tile([128, GROUPS * DIMS[li][1]], bf16, name=f"xr{li}",
                          tag=f"xr{li}") for li in range(3)]

        def allgather(li):
            nc.gpsimd.collective_compute(
                "AllGather", OP.bypass, replica_groups=[list(range(N_CORES))],
                ins=[sh_ev[li][:].opt()], outs=[tbl_ev[li][:].opt()])
            nc.gpsimd.collective_compute(
                "AllGather", OP.bypass, replica_groups=[list(range(N_CORES))],
                ins=[sh_od[li][:].opt()], outs=[tbl_od[li][:].opt()])

        # ---------------- layer-0 dense ----------------
        def dense0():
            d_in, d_out = DIMS[0]
            for g in range(GROUPS):
                lhs = sbuf.tile([d_in + 1, 128], f32, tag="dlhs")
                nc.sync.dma_start(out=lhs[:], in_=xT[:, g * 128:(g + 1) * 128])
                pm = psum.tile([128, ELEM], f32, tag="dns", space="PSUM")
                nc.tensor.matmul(out=pm[:], lhsT=lhs[:], rhs=W_t["Wl0"][:],
                                 start=True, stop=True)
                esc = sbuf.tile([128, ELEM], bf16, tag="desc")
                if g % 2 == 0:
                    nc.vector.tensor_copy(out=esc[:], in_=pm[:])
                else:
                    nc.scalar.copy(out=esc[:], in_=pm[:])
                r0 = g * 64
                nc.sync.dma_start(out=sh_ev[0][r0:r0 + 64, :], in_=esc[0:128:2, :])
                nc.sync.dma_start(out=sh_od[0][r0:r0 + 64, :], in_=esc[1:128:2, :])
                pm2 = psum.tile([128, d_out], f32, tag="dns", space="PSUM")
                nc.tensor.matmul(out=pm2[:], lhsT=lhs[:], rhs=W_t["Wr0"][:],
                                 start=True, stop=True)
                if g % 2 == 0:
                    nc.scalar.copy(out=xr_sb[0][:, g * d_out:(g + 1) * d_out], in_=pm2[:])
                else:
                    nc.vector.tensor_copy(out=xr_sb[0][:, g * d_out:(g + 1) * d_out], in_=pm2[:])
            allgather(0)

        # ---------------- edge phase for layer li ----------------
        def edge_phase(li):
            d_in, d_out = DIMS[li]
            sc = SC[li]
            n_pos = n_pos_l[li]
            po_of_group = {}
            qn = [0]

            def finalize_group(g):
                po2 = po_of_group.pop(g)
                dcl = sbuf.tile([128, 1], f32, tag="dcl")
                nc.vector.tensor_scalar(out=dcl[:], in0=po2[:, d_out:d_out + 1],
                                        scalar1=1e-30, scalar2=None, op0=OP.max)
                rec = sbuf.tile([128, 1], f32, tag="rec")
                nc.vector.reciprocal(out=rec[:], in_=dcl[:])
                hg = sbuf.tile([128, POOL_ELEM if li == 2 else d_out], f32, tag="hg")
                nc.vector.scalar_tensor_tensor(out=hg[:, :d_out], in0=po2[:, :d_out],
                                               scalar=rec[:], in1=W_t[f"invs{li}"][:],
                                               op0=OP.mult, op1=OP.mult)
                nc.vector.tensor_tensor(out=hg[:, :d_out], in0=hg[:, :d_out],
                                        in1=W_t[f"b{li}"][:], op=OP.add)
                if li < 2:
                    # relu, transpose, fused dense for layer li+1
                    d_next = DIMS[li + 1][1]
                    h2 = sbuf.tile([128, d_out], f32, tag="hrelu")
                    nc.scalar.activation(out=h2[:], in_=hg[:, :d_out], func=AF.Relu)
                    ptb = psum.tile([d_out, 128], f32, tag="aux", space="PSUM")
                    nc.tensor.transpose(out=ptb[:], in_=h2[:], identity=identf[:])
                    hTe = sbuf.tile([d_out + 1, 128], f32, tag="hTe")
                    nc.vector.memset(hTe[:], 1.0)
                    nc.vector.tensor_copy(out=hTe[:d_out, :], in_=ptb[:])
                    pm = psum.tile([128, ELEM], f32, tag="dns", space="PSUM")
                    nc.tensor.matmul(out=pm[:], lhsT=hTe[:], rhs=W_t[f"Wl{li + 1}"][:],
                                     start=True, stop=True)
                    esc = sbuf.tile([128, ELEM], bf16, tag="desc")
                    if g % 2 == 0:
                        nc.vector.tensor_copy(out=esc[:], in_=pm[:])
                    else:
                        nc.scalar.copy(out=esc[:], in_=pm[:])
                    r0 = g * 64
                    nc.sync.dma_start(out=sh_ev[li + 1][r0:r0 + 64, :], in_=esc[0:128:2, :])
                    nc.sync.dma_start(out=sh_od[li + 1][r0:r0 + 64, :], in_=esc[1:128:2, :])
                    pm2 = psum.tile([128, d_next], f32, tag="dns", space="PSUM")
                    nc.tensor.matmul(out=pm2[:], lhsT=hTe[:], rhs=W_t[f"Wr{li + 1}"][:],
                                     start=True, stop=True)
                    if g % 2 == 0:
                        nc.scalar.copy(out=xr_sb[li + 1][:, g * d_next:(g + 1) * d_next],
                                       in_=pm2[:])
                    else:
                        nc.vector.tensor_copy(out=xr_sb[li + 1][:, g * d_next:(g + 1) * d_next],
                                              in_=pm2[:])
                else:
                    nc.vector.memset(hg[:, d_out:d_out + 1], 1.0)
                    nc.vector.memset(hg[:, d_out + 1:], 0.0)
                    nc.sync.dma_start(out=h3_sh[g * 128:(g + 1) * 128, :], in_=hg[:])

            for (c0, nB, runs) in batches:
                # 1) gather: one SWDGE call per merged same-parity run
                slab = sbgat.tile([128, cap_b * ELEM], bf16, tag="slab")
                for (cs, n, p) in runs:
                    boff = cs - c0
                    tbl = tbl_ev[li] if p == 0 else tbl_od[li]
                    nc.gpsimd.dma_gather(
                        out_ap=slab[:, boff * ELEM:(boff + n) * ELEM]
                        .rearrange("q (c e) -> q c e", e=ELEM),
                        in_ap=tbl[:], idxs_ap=idx_t[:, cs * 8:(cs + n) * 8],
                        num_idxs=n * 128, num_idxs_reg=n * 128,
                        elem_size=ELEM, single_packet=False, queue_num=qn[0] % 4)
                    qn[0] += 1
                # 2) batched M2 one-hot [node, edge]: DMA partition-broadcast
                #    of drel_row, then in-place is_equal against iota_col
                m2 = sbslab.tile([128, cap_b * 128], bf16, tag="m2")
                nc.sync.dma_start(
                    out=m2[:, :nB * 128],
                    in_=drel_row[0:1, c0 * 128:(c0 + nB) * 128]
                    .to_broadcast([128, nB * 128]))
                nc.vector.tensor_scalar(
                    out=m2[:, :nB * 128], in0=m2[:, :nB * 128],
                    scalar1=iota_col[:], scalar2=None, op0=OP.is_equal)
                # 3) m' + Prelu + reduces per 4-chunk sub-slab
                eeb = sbuf.tile([128, cap_b], f32, tag="eeb")
                for s0 in range(0, nB, 4):
                    cn = min(4, nB - s0)
                    pm = psum.tile([128, 4 * d_out], f32, tag="pm", space="PSUM")
                    for j in range(cn):
                        c = c0 + s0 + j
                        g = chunk_grp[c]
                        nc.tensor.matmul(
                            out=pm[:, j * d_out:(j + 1) * d_out],
                            lhsT=m2[:, (s0 + j) * 128:(s0 + j + 1) * 128],
                            rhs=xr_sb[li][:, g * d_out:(g + 1) * d_out],
                            start=True, stop=False)
                        nc.tensor.matmul(
                            out=pm[:, j * d_out:(j + 1) * d_out],
                            lhsT=ident[:],
                            rhs=slab[:, (s0 + j) * ELEM:(s0 + j) * ELEM + d_out],
                            start=False, stop=True)
                    Ls = sbuf.tile([128, 4 * d_out], bf16, tag="lslab")
                    pv = pm[:, :cn * d_out].rearrange("q (c d) -> q c d", d=d_out)
                    Lw = Ls[:, :cn * d_out].rearrange("q (c d) -> q c d", d=d_out)
                    if n_pos > 0:
                        nc.scalar.activation(out=Lw[:, :, :n_pos], in_=pv[:, :, :n_pos],
                                             func=AF.Prelu, alpha=NEG_SLOPE)
                    if n_pos < d_out:
                        nc.scalar.activation(out=Lw[:, :, n_pos:], in_=pv[:, :, n_pos:],
                                             func=AF.Prelu, alpha=1.0 / NEG_SLOPE)
                    nc.vector.tensor_reduce(out=eeb[:, s0:s0 + cn], in_=Lw,
                                            axis=mb.AxisListType.X, op=OP.add)
                pe = sbuf.tile([128, cap_b], bf16, tag="pe")
                nc.scalar.activation(out=pe[:, :nB], in_=eeb[:, :nB], func=AF.Exp)
                # 4) batched M1p = (iota == drel) * p   [edge, node]
                m1p = sbslab.tile([128, cap_b * 128], bf16, tag="m1p")
                iota_b = manual_ap(iota16[:], [list(iota16[:].ap[0]), [0, nB],
                                                 list(iota16[:].ap[1])])
                nc.vector.tensor_tensor(
                    out=m1p[:, :nB * 128].rearrange("q (c n) -> q c n", n=128),
                    in0=drelc[:, c0:c0 + nB].to_broadcast([128, nB, 128]),
                    in1=iota_b, op=OP.is_equal)
                nc.vector.tensor_tensor(
                    out=m1p[:, :nB * 128].rearrange("q (c n) -> q c n", n=128),
                    in0=m1p[:, :nB * 128].rearrange("q (c n) -> q c n", n=128),
                    in1=pe[:, :nB].to_broadcast([128, nB, 128]), op=OP.mult)
                # 5) scatter + finalize
                for b in range(nB):
                    c = c0 + b
                    g = chunk_grp[c]
                    if c == first_chunk[g]:
                        po2 = psumo.tile([128, sc], f32, tag="pout",
                                         space="PSUM", name=f"po_{li}_{g}")
                        po_of_group[g] = po2
                    nc.tensor.matmul(
                        out=po_of_group[g][:],
                        lhsT=m1p[:, b * 128:(b + 1) * 128],
                        rhs=slab[:, b * ELEM:b * ELEM + sc],
                        start=(c == first_chunk[g]), stop=(c == last_chunk[g]))
                    if c == last_chunk[g]:
                        finalize_group(g)

            if li < 2:
                allgather(li + 1)
            else:
                nc.gpsimd.collective_compute(
                    "AllGather", OP.bypass, replica_groups=[list(range(N_CORES))],
                    ins=[h3_sh[:].opt()], outs=[h3_full[:].opt()])

        # ---------------- emit ----------------
        dense0()
        for li in range(3):
            edge_phase(li)

        # ---------------- pooling + MLP (replicated on every core) ----------------
        poolrel_t = const.tile([128, max(n_jobs, 1)], f32)
        nc.sync.dma_start(out=poolrel_t[:], in_=poolrel[:])
        wfc1_t = const.tile([POOL_ELEM, 20], f32)
        nc.sync.dma_start(out=wfc1_t[:], in_=Wfc1[:])
        bfc1_t = const.tile([128, 20], f32)
        nc.sync.dma_start(out=bfc1_t[:], in_=bfc1[:])
        wfc2_t = const.tile([32, 1], f32)
        nc.sync.dma_start(out=wfc2_t[:], in_=Wfc2[:])
        bfc2_t = const.tile([128, 1], f32)
        nc.sync.dma_start(out=bfc2_t[:], in_=bfc2[:])

        from collections import defaultdict
        by_win = defaultdict(list)
        for j, (chunk, w) in enumerate(_POOL_JOBS_META):
            by_win[w].append((j, chunk))
        for w in range(n_win):
            jobs_w = by_win.get(w, [])
            pw = psumo.tile([POOL_ELEM, 128], f32, tag="pout", space="PSUM")
            if not jobs_w:
                o1 = sbuf.tile([128, 1], f32, tag="ow")
                nc.vector.memset(o1[:], 0.0)
                nc.sync.dma_start(out=out_t[w * 128:(w + 1) * 128, :], in_=o1[:])
                continue
            for jj, (j, chunk) in enumerate(jobs_w):
                hch = sbuf.tile([128, POOL_ELEM], f32, tag="hch")
                nc.sync.dma_start(out=hch[:], in_=h3_full[chunk * 128:(chunk + 1) * 128, :])
                oh = sbuf.tile([128, 128], f32, tag="poh")
                nc.vector.tensor_scalar(out=oh[:], in0=iota_row[:],
                                        scalar1=poolrel_t[:, j:j + 1], scalar2=None,
                                        op0=OP.is_equal)
                nc.tensor.matmul(out=pw[:], lhsT=hch[:], rhs=oh[:],
                                 start=(jj == 0), stop=(jj == len(jobs_w) - 1))
            pesc = sbuf.tile([POOL_ELEM, 128], f32, tag="pesc")
            nc.scalar.copy(out=pesc[:], in_=pw[:])
            ptw = psum.tile([128, POOL_ELEM], f32, tag="aux", space="PSUM")
            nc.tensor.transpose(out=ptw[:], in_=pesc[:], identity=identf[:POOL_ELEM, :POOL_ELEM])
            cnt_r = sbuf.tile([128, 1], f32, tag="cntr")
            nc.vector.tensor_scalar(out=cnt_r[:], in0=ptw[:, 30:31], scalar1=1.0,
                                    scalar2=None, op0=OP.max)
            rec = sbuf.tile([128, 1], f32, tag="prec")
            nc.vector.reciprocal(out=rec[:], in_=cnt_r[:])
            gt = sbuf.tile([128, POOL_ELEM], f32, tag="gt")
            nc.vector.tensor_scalar(out=gt[:], in0=ptw[:], scalar1=rec[:],
                                    scalar2=None, op0=OP.mult)
            nc.vector.memset(gt[:, 30:], 0.0)
            pgt = psum.tile([POOL_ELEM, 128], f32, tag="aux", space="PSUM")
            nc.tensor.transpose(out=pgt[:], in_=gt[:], identity=identf[:])
            gT = sbuf.tile([POOL_ELEM, 128], f32, tag="gTt")
            nc.vector.tensor_copy(out=gT[:], in_=pgt[:])
            pf1 = psum.tile([128, 20], f32, tag="aux", space="PSUM")
            nc.tensor.matmul(out=pf1[:], lhsT=gT[:], rhs=wfc1_t[:], start=True, stop=True)
            g1 = sbuf.tile([128, 32], f32, tag="g1")
            nc.vector.tensor_tensor(out=g1[:, :20], in0=pf1[:], in1=bfc1_t[:], op=OP.add)
            g1r = sbuf.tile([128, 32], f32, tag="g1r")
            nc.scalar.activation(out=g1r[:, :20], in_=g1[:, :20], func=AF.Relu)
            nc.vector.memset(g1r[:, 20:], 0.0)
            pg1 = psum.tile([32, 128], f32, tag="aux", space="PSUM")
            nc.tensor.transpose(out=pg1[:], in_=g1r[:], identity=identf[:])
            g1T = sbuf.tile([32, 128], f32, tag="g1T")
            nc.vector.tensor_copy(out=g1T[:], in_=pg1[:])
            pf2 = psum.tile([128, 1], f32, tag="aux", space="PSUM")
            nc.tensor.matmul(out=pf2[:], lhsT=g1T[:], rhs=wfc2_t[:], start=True, stop=True)
            ow = sbuf.tile([128, 1], f32, tag="ow")
            nc.vector.tensor_tensor(out=ow[:], in0=pf2[:], in1=bfc2_t[:], op=OP.add)
            nc.sync.dma_start(out=out_t[w * 128:(w + 1) * 128, :], in_=ow[:])

    return nc


_POOL_JOBS_META = []


# ---------------- top-level kernel ----------------
_CACHE = {}


def _install_ntff_hook():
    """Make trace=True work under axon when antenv.axon_hooks is missing."""
    import sys, types
    try:
        from antenv.axon_hooks import get_axon_ntff_profile_hook  # noqa
        return
    except ImportError:
        pass
    try:
        mod = types.ModuleType("antenv.axon_hooks")
        mod._hook = None
        mod.set_axon_ntff_profile_hook = lambda h: setattr(mod, "_hook", h)
        mod.get_axon_ntff_profile_hook = lambda: mod._hook
        try:
            import antenv
            antenv.axon_hooks = mod
        except ImportError:
            pkg = types.ModuleType("antenv")
            pkg.axon_hooks = mod
            sys.modules["antenv"] = pkg
        sys.modules["antenv.axon_hooks"] = mod
        from trn_agent_boot.trn_boot import _ntff_profile_via_ctypes
        mod.set_axon_ntff_profile_hook(_ntff_profile_via_ctypes('/opt/axon/libaxon_pjrt.so'))
        import concourse.bass_utils as bu
        bu.upload_artifacts = lambda d: str(d)
    except Exception as e:
        print("ntff hook install failed:", e)


def kernel(**inputs):
    global _POOL_JOBS_META
    x = np.asarray(inputs["x"], np.float32)
    ei = np.asarray(inputs["edge_index"], np.int64)
    batch = np.asarray(inputs["batch"], np.int64)

    loops = np.arange(N, dtype=np.int64)
    src = np.concatenate([ei[0], loops])
    dst = np.concatenate([ei[1], loops])

    sched = build_edge_schedule(src, dst)
    pool_jobs, n_win = build_pool_schedule(batch)
    pool_jobs.sort(key=lambda t: (t[1], t[0]))
    _POOL_JOBS_META = [(c, w) for (c, w, _) in pool_jobs]

    # ---- fold weights ----
    n_pos_l = []
    Wmats = {}
    prev_order, prev_s = None, None
    for li in range(3):
        d_in, d_out = DIMS[li]
        Wl, Wr, n_pos, order, cvec = fold_layer(inputs[f"Wl{li + 1}"], inputs[f"Wr{li + 1}"],
                                                 inputs[f"att{li + 1}"])
        if prev_order is not None:
            # previous layer's h comes out de-scaled; only the permutation folds
            Wl = Wl[:, prev_order].astype(np.float32)
            Wr = Wr[:, prev_order].astype(np.float32)
        n_pos_l.append(n_pos)
        b_t = np.asarray(inputs[f"b{li + 1}"], np.float32)[order].astype(np.float32)
        Wmats[f"invs{li}"] = np.tile((1.0 / cvec)[None, :], (128, 1)).astype(np.float32)
        # augmented: [d_in+1, ELEM] ; last input row = ones channel -> table ones col
        Wa = np.zeros((d_in + 1, ELEM), np.float32)
        Wa[:d_in, :d_out] = Wl.T
        Wa[d_in, d_out] = 1.0            # ones column for denominators
        Wra = np.zeros((d_in + 1, d_out), np.float32)
        Wra[:d_in, :] = Wr.T
        Wmats[f"Wl{li}"] = Wa
        Wmats[f"Wr{li}"] = Wra
        Wmats[f"b{li}"] = np.tile(b_t[None, :], (128, 1)).astype(np.float32)
        prev_order, prev_s = order, cvec

    # FC weights; fold layer-3 unscale/perm into W_fc1
    Wfc1 = np.asarray(inputs["W_fc1"], np.float32)          # [20, 30]
    Wfc1_f = Wfc1[:, prev_order].astype(np.float32)
    Wfc1_a = np.zeros((POOL_ELEM, 20), np.float32)
    Wfc1_a[:30, :] = Wfc1_f.T
    bfc1 = np.tile(np.asarray(inputs["b_fc1"], np.float32)[None, :], (128, 1))
    Wfc2_a = np.zeros((32, 1), np.float32)
    Wfc2_a[:20, 0] = np.asarray(inputs["W_fc2"], np.float32)[0]
    bfc2 = np.full((128, 1), float(np.asarray(inputs["b_fc2"], np.float32)[0]), np.float32)

    # ---- per-core inputs ----
    iota_row = np.broadcast_to(np.arange(128, dtype=np.float32), (128, 128)).copy()
    iota_col = np.arange(128, dtype=np.float32)[:, None].copy()
    poolrel = np.zeros((128, max(len(pool_jobs), 1)), np.float32)
    for j, (_, _, rel) in enumerate(pool_jobs):
        poolrel[:, j] = rel

    TC = sched["total_chunks"]
    in_maps = []
    for k in range(N_CORES):
        xTl = np.zeros((D_IN + 1, NPAD), np.float32)
        xTl[:D_IN, :NPC] = x[k * NPC:(k + 1) * NPC].T
        xTl[D_IN, :NPC] = 1.0
        drel_k = sched["dstrel"][k]
        in_maps.append({
            "xT": xTl,
            "idx": pack_idx(sched["idx_i16"][k]),
            "drel_col": drel_k.reshape(TC, 128).T.astype(ml_dtypes.bfloat16),
            "drel_row": drel_k[None, :].astype(ml_dtypes.bfloat16),
            "iota_row": iota_row, "iota_col": iota_col,
            "poolrel": poolrel,
            "Wfc1": Wfc1_a, "bfc1": bfc1, "Wfc2": Wfc2_a, "bfc2": bfc2,
            **{k2: v for k2, v in Wmats.items()},
        })

    key = "prog"
    if key not in _CACHE:
        nc = build_program(sched, n_pos_l, len(pool_jobs), n_win)
        nc.compile()
        split_excess_waits(nc)
        _CACHE[key] = nc
    nc = _CACHE[key]

    if os.environ.get("GAT_BUILD_ONLY", "0") == "1":
        return np.zeros((NUM_GRAPHS, 1), np.float32)
    trace = os.environ.get("GAT_TRACE", "0") == "1"
    if trace:
        _install_ntff_hook()
    r = run_bass_kernel_spmd(nc, in_maps, core_ids=list(range(N_CORES)), trace=trace)
    if trace and r.exec_time_ns is not None:
        print(f"HW exec time: {r.exec_time_ns} ns")
    out = r.results[0]["out"][:NUM_GRAPHS, :].astype(np.float32)
    return out
